# revision 18
# baseline (speedup 1.0000x reference)
"""MoE all-to-all token dispatch kernel for 8 Trainium2 NeuronCores.

Problem: out[d, t*K+k, :] = x[t, :] if expert_mapping[expert_indices[t, k]] == d
else 0, with B=4, S=4096, H=512, K=2, 64 experts, 8 devices.

Strategy: the output's leading device axis is sharded across the 8 cores —
core d produces out[d] = [T*K, H].  Only ~1/8 of each core's output rows are
nonzero, so each core gathers just its owned token rows from HBM into SBUF
(dma_gather) and writes them back to the owned output slots.

Two scatter paths exist with complementary bottlenecks, so rows are split
between them and both run concurrently:
  - paged_writeback (V path): plain-copy CME writes — cheap on the SDMA
    engines but expensive on the GpSimd Q7 descriptor generator (~36 ns/row:
    it streams 3 idx values per row one pop at a time, and batch > 128
    silently no-ops on this firmware so instructions stay small).
  - dma_scatter_add: cheap descriptor generation (wrapped 16-lane idx
    stream) but its CCE read-modify-write packets expose full HBM read
    latency per row (~500 ns per row on the engines).

paged_writeback's V path writes, for ptr p and in-page index j, a contiguous
d_head row at element offset (256p + 2j + 1)*H of its paged view; with the
output allocated one row early (real = dram[1:]) the view based at row 0
covers even real rows and the view based at row 1 covers odd real rows, so
each writeback chunk is ordered [even-row slots | odd-row slots].

The output DRAM buffer is pre-zeroed by the runtime, so untouched rows are
already correct; scatter-add padding targets row 0 with gathered zero rows
(xin carries an appended all-zero row), and writeback padding is skipped via
page_ptr = -1.  All instruction streams are static (one NEFF for all cores);
routing metadata is computed on the host.  Gathers run on SWDGE queue 0,
writebacks on queues 1/3, scatter-adds on queue 2, pipelined over a ring of
SBUF chunk buffers.
"""

import numpy as np

B, S, H, K = 4, 4096, 512, 2
T = B * S          # 16384 tokens
TK = T * K         # 32768 output rows per device
D = 8              # devices / NeuronCores
E = 64             # experts

ZROW = T           # index of the appended all-zero row in xin
CH = 1024          # slots per chunk
HB = CH // 2       # writeback slots per parity per chunk
WB = 128           # paged_writeback batch (>128 silently no-ops on this fw)
NBUF = 6           # chunk buffers in the SBUF ring

TRACE = False
LAST_EXEC_NS = None
LAST_RESULTS = None

_CACHE = {}


def _wrap_idxs16(vals: np.ndarray, n: int, pad: int) -> np.ndarray:
    """SWDGE wrapped int16 layout: element i at [i % 16, i // 16], `pad`
    tail, replicated across the 8 partition groups (128 partitions)."""
    arr = np.full(n, pad, np.int16)
    arr[: len(vals)] = vals.astype(np.int16)
    w = arr.reshape(n // 16, 16).T             # [16, n/16]
    return np.ascontiguousarray(np.tile(w, (8, 1)))  # [128, n/16]


def _build_module(nchw: int, nchs: int):
    from contextlib import ExitStack

    import concourse.bacc as bacc
    import concourse.mybir as mybir
    from concourse.library_config import attnmlp

    nch = nchw + nchs
    maxn = nch * CH
    nbc = CH // 128        # data columns per chunk (8)
    wc = CH // 16          # wrapped-idx16 columns per chunk (64)
    iwc = 6 * HB           # widx int32 columns per writeback chunk
    nwb = HB // WB         # batch-128 writebacks per parity per chunk (4)

    # alternate chunk types W,S,W,S,... then the remainder
    types = []
    for i in range(max(nchw, nchs)):
        if i < nchw:
            types.append("W")
        if i < nchs:
            types.append("S")

    nc = bacc.Bacc("TRN2", debug=False, num_swdge_queues=4)
    xin = nc.dram_tensor("xin", [T + 1, H], mybir.dt.float32,
                         kind="ExternalInput")
    sidx = nc.dram_tensor("sidx", [128, maxn // 16], mybir.dt.int16,
                          kind="ExternalInput")
    widx = nc.dram_tensor("widx", [128, max(nchw, 1) * iwc], mybir.dt.int32,
                          kind="ExternalInput")
    didx = nc.dram_tensor("didx", [128, max(nchs, 1) * wc], mybir.dt.int16,
                          kind="ExternalInput")
    out = nc.dram_tensor("out", [TK + 1, H], mybir.dt.float32,
                         kind="ExternalOutput")

    # paged views: [n_pages, 128, 2*d_head*page_size//128] = [128, 128, 1024]
    view_even = out[0:TK, :].rearrange("(p j t) h -> p j (t h)", j=128, t=2)
    view_odd = out[1:TK + 1, :].rearrange("(p j t) h -> p j (t h)", j=128, t=2)
    out_rows = out[1:TK + 1, :]          # scatter-add target (real rows)

    with (
        nc.Block() as block,
        nc.sbuf_tensor("data", [128, NBUF, nbc, H], mybir.dt.float32) as data,
        nc.sbuf_tensor("sidx_sb", [128, maxn // 16], mybir.dt.int16) as sidx_sb,
        nc.sbuf_tensor("widx_sb", [128, max(nchw, 1) * iwc],
                       mybir.dt.int32) as widx_sb,
        nc.sbuf_tensor("didx_sb", [128, max(nchs, 1) * wc],
                       mybir.dt.int16) as didx_sb,
        nc.semaphore("io") as io,
        nc.semaphore("wsem0") as wsem0,
        nc.semaphore("wsem1") as wsem1,
        nc.semaphore("ssem") as ssem,
        ExitStack() as stack,
    ):
        gsems = [stack.enter_context(nc.semaphore(f"g{i}"))  # noqa: ANT232
                 for i in range(min(NBUF, nch))]

        @block.gpsimd
        def _(gpsimd):
            gpsimd.load_library(attnmlp)
            gpsimd.dma_start(sidx_sb[:], sidx[:]).then_inc(io, 16)
            gpsimd.dma_start(widx_sb[:], widx[:]).then_inc(io, 16)
            gpsimd.dma_start(didx_sb[:], didx[:]).then_inc(io, 16)
            gpsimd.wait_ge(io, 48)

            def gather(c):
                gpsimd.dma_gather(
                    data[:, c % NBUF, :, :], xin[:],
                    sidx_sb[:, c * wc:(c + 1) * wc], CH, CH, H,
                    single_packet=False, queue_num=0,
                ).then_inc(gsems[c % NBUF], 16)

            for c in range(min(NBUF, nch)):
                gather(c)

            cw = cs = 0
            for c, ty in enumerate(types):
                gpsimd.wait_ge(gsems[c % NBUF], 16 * (c // NBUF + 1))
                if ty == "W":
                    base = cw * iwc
                    for w in range(nwb):
                        gpsimd.paged_writeback(
                            view_even, data[:, c % NBUF, w, :],
                            widx_sb[:, base + 3 * WB * w:
                                    base + 3 * WB * (w + 1)],
                            WB, 1, 128, H, "v", queue_num=1,
                        ).then_inc(wsem0, 16)
                        gpsimd.paged_writeback(
                            view_odd, data[:, c % NBUF, nbc // 2 + w, :],
                            widx_sb[:, base + 3 * HB + 3 * WB * w:
                                    base + 3 * HB + 3 * WB * (w + 1)],
                            WB, 1, 128, H, "v", queue_num=3,
                        ).then_inc(wsem1, 16)
                    cw += 1
                else:
                    gpsimd.dma_scatter_add(
                        out_rows, data[:, c % NBUF, :, :],
                        didx_sb[:, cs * wc:(cs + 1) * wc], CH, CH, H,
                        single_packet=False, queue_num=2,
                    ).then_inc(ssem, 16)
                    cs += 1
                if c + NBUF < nch:
                    # ring WAR: all writes of chunks <= c must land before
                    # gather c+NBUF reuses the buffer
                    gpsimd.wait_ge(wsem0, 16 * nwb * cw)
                    gpsimd.wait_ge(wsem1, 16 * nwb * cw)
                    gpsimd.wait_ge(ssem, 16 * cs)
                    gather(c + NBUF)
            gpsimd.wait_ge(wsem0, 16 * nwb * nchw)
            gpsimd.wait_ge(wsem1, 16 * nwb * nchw)
            gpsimd.wait_ge(ssem, 16 * nchs)

    nc.compile()
    return nc, types


def _prep_core(dst: np.ndarray, nchw: int, nchs: int, types):
    """Per-core host prep: slot ordering + gather/writeback/scatter idxs."""
    evens = dst[dst % 2 == 0]
    odds = dst[dst % 2 == 1]
    wbe, sce = evens[: nchw * HB], evens[nchw * HB:]
    wbo, sco = odds[: nchw * HB], odds[nchw * HB:]
    scat = np.sort(np.concatenate([sce, sco]))

    nch = nchw + nchs
    src16 = np.full(nch * CH, ZROW, np.int16)
    widx = np.full(max(nchw, 1) * 6 * HB, -1, np.int32)
    didx = np.full(max(nchs, 1) * CH, -1, np.int64)

    cw = cs = 0
    for c, ty in enumerate(types):
        base = c * CH
        if ty == "W":
            es = wbe[cw * HB:(cw + 1) * HB]
            os_ = wbo[cw * HB:(cw + 1) * HB]
            src16[base:base + len(es)] = es // K
            src16[base + HB:base + HB + len(os_)] = os_ // K
            wb = cw * 6 * HB
            for off, rows, sub in ((0, es, 0), (3 * HB, os_, 1)):
                m = (rows - sub) // 2
                ptr1 = np.full(HB, -1, np.int32)
                pidx = np.zeros(HB, np.int32)
                ptr1[: len(rows)] = m // 128
                pidx[: len(rows)] = m % 128
                for w in range(HB // WB):
                    blk = wb + off + 3 * WB * w
                    widx[blk:blk + WB] = ptr1[WB * w:WB * (w + 1)]
                    widx[blk + WB:blk + 2 * WB] = -1
                    widx[blk + 2 * WB:blk + 3 * WB] = pidx[WB * w:WB * (w + 1)]
            cw += 1
        else:
            rows = scat[cs * CH:(cs + 1) * CH]
            src16[base:base + len(rows)] = rows // K
            blk = cs * CH
            didx[blk:blk + CH] = 0                      # pad: zero-add row 0
            didx[blk:blk + len(rows)] = rows
            cs += 1
    return src16, widx, didx


def kernel(input_tensor, expert_indices, expert_mapping):
    global LAST_EXEC_NS, LAST_RESULTS
    from concourse.bass_utils import run_bass_kernel_spmd

    x = np.zeros((T + 1, H), dtype=np.float32)
    x[:T] = np.asarray(input_tensor, dtype=np.float32).reshape(T, H)
    idx = np.asarray(expert_indices, dtype=np.int32).reshape(-1)
    emap = np.asarray(expert_mapping, dtype=np.int32)
    owner = emap[idx]                                  # [T*K], slot r = t*K+k

    dsts = [np.nonzero(owner == d)[0] for d in range(D)]
    maxn = max(len(v) for v in dsts)
    # ~half the rows through each write path (writeback chunks consume up to
    # HB rows per parity per chunk)
    nchw = max(1, round(maxn / 2 / CH))
    nchs = max(
        -(-int((len(v) - min((v % 2 == 0).sum(), nchw * HB)
                - min((v % 2 == 1).sum(), nchw * HB))) // CH)
        for v in dsts
    )
    nchs = max(nchs, 1)

    if (nchw, nchs) not in _CACHE:
        _CACHE[(nchw, nchs)] = _build_module(nchw, nchs)
    nc, types = _CACHE[(nchw, nchs)]

    in_maps = []
    for d in range(D):
        src16, widx, didx = _prep_core(dsts[d], nchw, nchs, types)
        in_maps.append({
            "xin": x,
            "sidx": _wrap_idxs16(src16, len(src16), pad=ZROW),
            "widx": np.ascontiguousarray(np.tile(widx[None, :], (128, 1))),
            "didx": _wrap_idxs16(didx, len(didx), pad=0),
        })

    res = run_bass_kernel_spmd(nc, in_maps, list(range(D)), trace=TRACE)
    if TRACE:
        LAST_EXEC_NS = res.exec_time_ns
        LAST_RESULTS = res
    return np.stack([res.results[d]["out"][1:] for d in range(D)], axis=0)


# revision 19
# speedup vs baseline: 1.5680x; 1.5680x over previous
"""MoE all-to-all token dispatch kernel for 8 Trainium2 NeuronCores.

Problem: out[d, t*K+k, :] = x[t, :] if expert_mapping[expert_indices[t, k]] == d
else 0, with B=4, S=4096, H=512, K=2, 64 experts, 8 devices.

Strategy: the output's leading device axis is sharded across the 8 cores —
core d produces out[d] = [T*K, H].  Only ~1/8 of each core's output rows are
nonzero (each (t, k) slot is owned by exactly one device), so instead of
writing the dense 64 MiB slab, each core gathers just its owned token rows
from HBM into SBUF (dma_gather) and scatter-adds them into the owned slots of
the output (dma_scatter_add).  The output DRAM buffer is pre-zeroed by the
runtime (run_bass_kernel_spmd zero-fills/donates ExternalOutput buffers), so
untouched rows are already correct.

Routing metadata (which rows each core owns) is computed on the host from
expert_indices/expert_mapping and passed per-core as int16 index tensors.
Per-core counts are padded to a common multiple-of-CH maxn with all-valid
indices: padded gather slots read a zero row appended to xin (index T), and
padded scatter slots add those zeros to out row 0 — a no-op.  This keeps the
instruction stream fully static (one NEFF for all 8 cores, no runtime count
registers).

Work is pipelined chunk by chunk: gathers run on SWDGE queues 0/2,
scatter-adds on queues 1/3, so the SDMA engines interleave both streams.
"""

import numpy as np

B, S, H, K = 4, 4096, 512, 2
T = B * S          # 16384 tokens
TK = T * K         # 32768 output rows per device
D = 8              # devices / NeuronCores
E = 64             # experts

ZROW = T           # index of the appended all-zero row in xin
CH = 512           # slots per chunk (multiple of 128)

TRACE = False
LAST_EXEC_NS = None
LAST_RESULTS = None

_CACHE = {}


def _wrap_idxs16(vals: np.ndarray, maxn: int, pad: int) -> np.ndarray:
    """SWDGE wrapped int16 layout: element i at [i % 16, i // 16], `pad`
    tail, replicated across the 8 partition groups (128 partitions)."""
    arr = np.full(maxn, pad, np.int16)
    arr[: len(vals)] = vals.astype(np.int16)
    w = arr.reshape(maxn // 16, 16).T          # [16, maxn/16]
    return np.ascontiguousarray(np.tile(w, (8, 1)))  # [128, maxn/16]


def _build_module(maxn: int):
    from contextlib import ExitStack

    import concourse.bacc as bacc
    import concourse.mybir as mybir
    from concourse.library_config import mlp

    assert maxn % CH == 0
    nb = maxn // 128
    nch = maxn // CH
    nbc = CH // 128        # data columns per chunk
    wc = CH // 16          # wrapped-idx columns per chunk

    nc = bacc.Bacc("TRN2", debug=False, num_swdge_queues=4)
    xin = nc.dram_tensor("xin", [T + 1, H], mybir.dt.float32,
                         kind="ExternalInput")
    sidx = nc.dram_tensor("sidx", [128, maxn // 16], mybir.dt.int16,
                          kind="ExternalInput")
    didx = nc.dram_tensor("didx", [128, maxn // 16], mybir.dt.int16,
                          kind="ExternalInput")
    out = nc.dram_tensor("out", [TK, H], mybir.dt.float32,
                         kind="ExternalOutput")

    with (
        nc.Block() as block,
        nc.sbuf_tensor("data", [128, nb, H], mybir.dt.float32) as data,
        nc.sbuf_tensor("sidx_sb", [128, maxn // 16], mybir.dt.int16) as sidx_sb,
        nc.sbuf_tensor("didx_sb", [128, maxn // 16], mybir.dt.int16) as didx_sb,
        nc.semaphore("io") as io,
        nc.semaphore("ssem0") as ssem0,
        nc.semaphore("ssem1") as ssem1,
        ExitStack() as stack,
    ):
        gsems = [stack.enter_context(nc.semaphore(f"g{c}"))  # noqa: ANT232
                 for c in range(nch)]

        @block.gpsimd
        def _(gpsimd):
            gpsimd.load_library(mlp)
            gpsimd.dma_start(sidx_sb[:], sidx[:]).then_inc(io, 16)
            gpsimd.dma_start(didx_sb[:], didx[:]).then_inc(io, 16)
            gpsimd.wait_ge(io, 32)
            # Enqueue every gather chunk up front (queues 0/2); the SDMA
            # engines drain them while scatters (queues 1/3) run behind.
            for c in range(nch):
                gpsimd.dma_gather(
                    data[:, c * nbc:(c + 1) * nbc, :], xin[:],
                    sidx_sb[:, c * wc:(c + 1) * wc], CH, CH, H,
                    single_packet=False, queue_num=(c % 2) * 2,
                ).then_inc(gsems[c], 16)
            for c in range(nch):
                gpsimd.wait_ge(gsems[c], 16)
                gpsimd.dma_scatter_add(
                    out[:], data[:, c * nbc:(c + 1) * nbc, :],
                    didx_sb[:, c * wc:(c + 1) * wc], CH, CH, H,
                    single_packet=False, queue_num=(c % 2) * 2 + 1,
                ).then_inc(ssem0 if c % 2 == 0 else ssem1, 16)
            gpsimd.wait_ge(ssem0, 16 * ((nch + 1) // 2))
            gpsimd.wait_ge(ssem1, 16 * (nch // 2))

    nc.compile()
    return nc


def kernel(input_tensor, expert_indices, expert_mapping):
    global LAST_EXEC_NS, LAST_RESULTS
    from concourse.bass_utils import run_bass_kernel_spmd

    x = np.zeros((T + 1, H), dtype=np.float32)
    x[:T] = np.asarray(input_tensor, dtype=np.float32).reshape(T, H)
    idx = np.asarray(expert_indices, dtype=np.int32).reshape(-1)
    emap = np.asarray(expert_mapping, dtype=np.int32)
    owner = emap[idx]                                  # [T*K], slot r = t*K+k

    dsts = [np.nonzero(owner == d)[0] for d in range(D)]
    maxn = -(-max(len(v) for v in dsts) // CH) * CH

    if maxn not in _CACHE:
        _CACHE[maxn] = _build_module(maxn)
    nc = _CACHE[maxn]

    in_maps = []
    for d in range(D):
        dst = dsts[d]
        src = dst // K
        in_maps.append({
            "xin": x,
            "sidx": _wrap_idxs16(src, maxn, pad=ZROW),
            "didx": _wrap_idxs16(dst, maxn, pad=0),
        })

    res = run_bass_kernel_spmd(nc, in_maps, list(range(D)), trace=TRACE)
    if TRACE:
        LAST_EXEC_NS = res.exec_time_ns
        LAST_RESULTS = res
    return np.stack([res.results[d]["out"] for d in range(D)], axis=0)
